# revision 1
# baseline (speedup 1.0000x reference)
"""Low-rank linear attention (causal, elu+1 feature map) on 8 trn2 cores.

Sharding: core = 2*b + h  (batch b in 0..3, sequence half h in 0..1).
Each core computes out[b, h*2048:(h+1)*2048, :].  Second-half cores
recompute the running K^T.V state over their 2048-token prefix on device
(sel input scales the prefix contribution to zero on first-half cores so
one SPMD program serves all 8 cores).

Phase A: all projections (prefix [V|K] + state accumulation, main
[Q^T;K^T] and [V|K]) — dependency-free dense PE stream.
Phase B: per-chunk causal attention (scores, num/den, state chain,
output projection with the 1/den fold into the PSUM eviction).

Shapes (hardcoded): B=4, S=4096, D=1024, K=64.  L = S/2 = 2048 tokens
per core, processed in 16 chunks of C=128.
"""

import numpy as np

B, S, D, K = 4, 4096, 1024, 64
L = S // 2          # tokens per core (main), also prefix length
C = 128             # chunk (tokens)
G = 512             # token group for P1 matmuls
NCHUNK = L // C     # 16
NGRP = L // G       # 4
NDC = D // 128      # 8 contraction chunks

_cache = {}


def _build_nc():
    import concourse.bacc as bacc
    import concourse.tile as tile
    from concourse import mybir

    f32 = mybir.dt.float32
    bf16 = mybir.dt.bfloat16
    AF = mybir.ActivationFunctionType
    Alu = mybir.AluOpType

    nc = bacc.Bacc()

    xtm = nc.declare_dram_parameter("xtm", [D, L], bf16, isOutput=False)
    xtp = nc.declare_dram_parameter("xtp", [D, L], bf16, isOutput=False)
    wcat = nc.declare_dram_parameter("wcat", [128, 2 * D + C], bf16, isOutput=False)
    wot = nc.declare_dram_parameter("wot", [K, D], bf16, isOutput=False)
    sel = nc.declare_dram_parameter("sel", [C, 1], f32, isOutput=False)
    out = nc.declare_dram_parameter("out", [L, D], f32, isOutput=True)

    with tile.TileContext(nc) as tc:
        with (
            tc.tile_pool(name="consts", bufs=1) as consts,
            tc.tile_pool(name="xm", bufs=1) as xm_pool,
            tc.tile_pool(name="xp", bufs=1) as xp_pool,
            tc.tile_pool(name="small", bufs=4) as small,
            tc.tile_pool(name="vko", bufs=2 * NCHUNK + 1) as vko_pool,
            tc.tile_pool(name="qk", bufs=NCHUNK + 1) as qk_pool,
            tc.tile_pool(name="tmp", bufs=4) as tmp_pool,
            tc.tile_pool(name="ostage", bufs=3) as ostage_pool,
            tc.tile_pool(name="state_pool", bufs=1, space="PSUM") as state_pool,
        ):
            # ---- constants ----
            wcat_sb = consts.tile([128, 2 * D + C], bf16, tag="wcat")
            nc.sync.dma_start(out=wcat_sb, in_=wcat[:, :])
            wqk_sb = [wcat_sb[:, d * 128:(d + 1) * 128] for d in range(NDC)]
            wvk_sb = [wcat_sb[:, D + d * 128:D + (d + 1) * 128] for d in range(NDC)]
            mask_sb = wcat_sb[:, 2 * D:2 * D + C]
            wot_sb = consts.tile([K, D], bf16, tag="wot")
            nc.sync.dma_start(out=wot_sb, in_=wot[:, :])
            sel_sb = consts.tile([C, 1], f32, tag="sel")
            nc.sync.dma_start(out=sel_sb, in_=sel[:, :])
            ones1_sb = consts.tile([1, 1], bf16, tag="ones1")
            nc.vector.memset(ones1_sb, 1.0)
            onesr = consts.tile([1, G], bf16, tag="onesr")
            nc.vector.memset(onesr, 1.0)
            vkbias = consts.tile([1, 2 * K], bf16, tag="vkbias")
            nc.vector.memset(vkbias[:, 0:K], 0.0)
            nc.vector.memset(vkbias[:, K:2 * K], 1.0)
            biasm1 = consts.tile([128, 1], f32, tag="biasm1")
            nc.vector.memset(biasm1, -1.0)

            # ---- x tiles (resident); DMA'd in group sections so early
            # chunks' operands land first and PE ramps immediately ----
            xp_all = []
            xm_all = []
            for d in range(NDC):
                xp_t = xp_pool.tile([128, L], bf16, tag=f"xp{d}")
                xp_all.append(xp_t)
                xm_t = xm_pool.tile([128, L], bf16, tag=f"xm{d}")
                xm_all.append(xm_t)
            for g in range(NGRP):
                gs = slice(g * G, (g + 1) * G)
                for d in range(NDC):
                    nc.sync.dma_start(out=xp_all[d][:, gs],
                                      in_=xtp[d * 128:(d + 1) * 128, gs])
                    nc.sync.dma_start(out=xm_all[d][:, gs],
                                      in_=xtm[d * 128:(d + 1) * 128, gs])

            # running state [K, K+1]: cols 0:K = S[k,m], col K = k_sum.
            state_ps = state_pool.tile([K, 1 + K], f32)

            # =============== PHASE A: projections ===============
            qTs, kTs, vkos = [], [], []
            with (
                tc.tile_pool(name="p1_ps", bufs=3, space="PSUM") as p1_pool,
                tc.tile_pool(name="p2_ps", bufs=4, space="PSUM") as p2_pool,
            ):
                def project_vk(xg, sl):
                    """token-major [V | ones | elu(K)+1] sbuf tile for a chunk."""
                    pp = p2_pool.tile([C, 2 * K], f32, tag="p2")
                    for d in range(NDC):
                        nc.tensor.matmul(pp, xg[d][:, sl], wvk_sb[d],
                                         start=(d == 0), stop=False)
                    nc.tensor.matmul(pp, onesr[:, 0:C], vkbias,
                                     start=False, stop=True)
                    vko = vko_pool.tile([C, 2 * K + 1], bf16, tag="vko")
                    nc.vector.memset(vko[:, K:K + 1], 1.0)
                    nc.vector.tensor_copy(vko[:, 0:K], pp[:, 0:K])
                    eu = tmp_pool.tile([C, K], f32, tag="eu")
                    nc.scalar.activation(eu, pp[:, K:2 * K], AF.Exp, bias=biasm1)
                    nc.vector.scalar_tensor_tensor(
                        vko[:, K + 1:2 * K + 1], eu, 1.0, pp[:, K:2 * K],
                        op0=Alu.min, op1=Alu.max)
                    return vko

                # interleave prefix + main by group so compute follows
                # the DMA section order
                for g in range(NGRP):
                    for c4 in range(G // C):
                        ci = g * (G // C) + c4
                        sl = slice(ci * C, (ci + 1) * C)
                        vko = project_vk(xp_all, sl)
                        vks = vko_pool.tile([C, K + 1], bf16, tag="vks")
                        nc.vector.tensor_scalar_mul(vks, vko[:, 0:K + 1], sel_sb)
                        nc.tensor.matmul(state_ps, vko[:, K + 1:2 * K + 1], vks,
                                         start=(ci == 0), stop=False,
                                         skip_group_check=True)
                    xg = [t[:, g * G:(g + 1) * G] for t in xm_all]
                    p1g = p1_pool.tile([2 * K, G], f32, tag="p1")
                    for d in range(NDC):
                        nc.tensor.matmul(p1g, wqk_sb[d], xg[d],
                                         start=(d == 0), stop=False)
                    nc.tensor.matmul(
                        p1g, ones1_sb[:, 0:1].to_broadcast((1, 2 * K)),
                        onesr, start=False, stop=True)
                    for c4 in range(G // C):
                        sl = slice(c4 * C, (c4 + 1) * C)
                        e1 = tmp_pool.tile([2 * K, C], f32, tag="e1")
                        nc.scalar.activation(e1, p1g[:, sl], AF.Exp, bias=biasm1)
                        qT = qk_pool.tile([K, C], bf16, tag="qT")
                        kT = qk_pool.tile([K, C], bf16, tag="kT")
                        nc.vector.scalar_tensor_tensor(
                            qT, e1[0:K, :], 1.0, p1g[0:K, sl],
                            op0=Alu.min, op1=Alu.max)
                        nc.vector.scalar_tensor_tensor(
                            kT, e1[K:2 * K, :], 1.0, p1g[K:2 * K, sl],
                            op0=Alu.min, op1=Alu.max)
                        qTs.append(qT)
                        kTs.append(kT)
                        vkos.append(project_vk(
                            xm_all,
                            slice(g * G + c4 * C, g * G + (c4 + 1) * C)))

            # sbuf copy of the running state used as matmul lhsT
            ks_sb = small.tile([K, 1 + K], bf16, tag="ks")
            nc.scalar.copy(ks_sb, state_ps)

            # =============== PHASE B: attention ===============
            with tc.tile_pool(name="atnd_ps", bufs=7, space="PSUM") as atnd_pool:
                for ci in range(NCHUNK):
                    qT, kT, vko = qTs[ci], kTs[ci], vkos[ci]
                    # intra-chunk scores A^T[t, s], causal mask
                    at = atnd_pool.tile([C, C], f32, tag="atnd")
                    nc.tensor.matmul(at, kT, qT, start=True, stop=True)
                    atm = tmp_pool.tile([C, C], bf16, tag="atm")
                    nc.vector.tensor_tensor(atm, at, mask_sb, Alu.mult)
                    # [num^T ; den] via lhsT-packed pair
                    nd = atnd_pool.tile([1 + K, C], f32, tag="atnd")
                    nc.tensor.matmul(nd, vko[:, 0:K + 1], atm,
                                     start=True, stop=False)
                    nc.tensor.matmul(nd, ks_sb, qT, start=False, stop=True)
                    # state update + refresh ks_sb
                    nc.tensor.matmul(state_ps, vko[:, K + 1:2 * K + 1],
                                     vko[:, 0:K + 1],
                                     start=False, stop=(ci == NCHUNK - 1),
                                     skip_group_check=True)
                    nc.scalar.copy(ks_sb, state_ps)
                    # reciprocal of den (transpose via 1-row matmul)
                    den_b = small.tile([1, C], bf16, tag="den")
                    nc.vector.tensor_scalar_add(den_b, nd[K:K + 1, :], 1e-6)
                    dtp = atnd_pool.tile([C, 1], f32, tag="atnd")
                    nc.tensor.matmul(dtp, den_b, ones1_sb, start=True, stop=True)
                    recip = small.tile([C, 1], f32, tag="recip")
                    nc.vector.reciprocal(recip, dtp)
                    # output projection; divide by den on PSUM eviction
                    attn = qk_pool.tile([K, C], bf16, tag="attn")
                    nc.vector.tensor_copy(attn, nd[0:K, :])
                    ost = ostage_pool.tile([C, D], f32, tag="ost")
                    for h2 in range(2):
                        op = atnd_pool.tile([C, D // 2], f32, tag="atnd")
                        nc.tensor.matmul(
                            op, attn, wot_sb[:, h2 * 512:(h2 + 1) * 512],
                            start=True, stop=True)
                        nc.scalar.activation(
                            ost[:, h2 * 512:(h2 + 1) * 512], op,
                            AF.Copy, scale=recip)
                    nc.sync.dma_start(out=out[ci * C:(ci + 1) * C, :], in_=ost)

    nc.compile()
    worst = []
    for fn in nc.m.functions:
        for blk in fn.blocks:
            for inst in blk.instructions:
                n = len(inst.sync_info.on_wait) if inst.sync_info else 0
                if n > 1 and type(inst).__name__ == "InstMatmult":
                    worst.append((inst.name, n))
    if worst:
        raise RuntimeError(f"matmuls with >1 wait after lowering: {worst}")
    return nc


def _prep_inputs(x, Wq, Wk, Wv, Wo):
    import ml_dtypes

    bf16 = ml_dtypes.bfloat16
    wqk = np.concatenate([Wq.T, Wk.T], axis=1)                # [D, 2K]
    wvk = np.concatenate([Wv.T, Wk.T], axis=1)                # [D, 2K]
    mask = np.triu(np.ones((C, C), np.float32))               # keep t <= s
    wcat = np.concatenate(
        [wqk[d * 128:(d + 1) * 128, :] for d in range(NDC)]
        + [wvk[d * 128:(d + 1) * 128, :] for d in range(NDC)]
        + [mask],
        axis=1,
    ).astype(bf16)
    wot = np.ascontiguousarray(Wo.T).astype(bf16)             # [K, D]
    zeros_x = np.zeros((D, L), dtype=bf16)
    in_maps = []
    for core in range(8):
        b, h = core // 2, core % 2
        xb = np.ascontiguousarray(x[b].astype(bf16).T)        # [D, S]
        m = {
            "xtm": np.ascontiguousarray(xb[:, h * L:(h + 1) * L]),
            "xtp": np.ascontiguousarray(xb[:, 0:L]) if h else zeros_x,
            "wcat": wcat,
            "wot": wot,
            "sel": np.full((C, 1), float(h), np.float32),
        }
        in_maps.append(m)
    return in_maps


def _run(inputs, trace=False):
    from concourse.bass_utils import run_bass_kernel_spmd

    if "nc" not in _cache:
        _cache["nc"] = _build_nc()
    nc = _cache["nc"]
    in_maps = _prep_inputs(
        np.asarray(inputs["x"], np.float32),
        np.asarray(inputs["Wq"], np.float32),
        np.asarray(inputs["Wk"], np.float32),
        np.asarray(inputs["Wv"], np.float32),
        np.asarray(inputs["Wo"], np.float32),
    )
    res = run_bass_kernel_spmd(nc, in_maps, list(range(8)), trace=trace)
    out = np.empty((B, S, D), np.float32)
    for core in range(8):
        b, h = core // 2, core % 2
        out[b, h * L:(h + 1) * L, :] = res.results[core]["out"]
    return out, res


def kernel(**inputs) -> np.ndarray:
    out, _ = _run(inputs, trace=False)
    return out



# revision 6
# speedup vs baseline: 1.2060x; 1.2060x over previous
"""Low-rank linear attention (causal, elu+1 feature map) on 8 trn2 cores.

Sharding: core = 2*b + h  (batch b in 0..3, sequence half h in 0..1).
Each core computes out[b, h*2048:(h+1)*2048, :].  Second-half cores
recompute the running K^T.V state over their 2048-token prefix on device
(prefix V contributions are scaled by sel=h so one SPMD program serves
all 8 cores).

Structure (v2):
 - prefix: token-major [K|V] projection per 128-token chunk; rank-128
   state updates accumulate sum_t k_t (x) [v_t/16, 1/16] in PSUM.
 - main: K-major [Q;K] projection in 512-token groups (weights moving,
   M=128 packed) + token-major V-only projection (free dim 64) +
   per-chunk PE transpose of kT to get token-major k.
 - The elu+1 feature map is computed as max(z+1, min(exp(z), 1)) on
   scalar+vector engines (no bias matmuls).
 - V carries a 1/16 scale so num and den stay scaled consistently
   (ratio unchanged); output written to DRAM as bf16.
 - phase B: per-chunk running-state snapshots (no WAR serialization),
   den transposed via 1-row matmul, output projection evictions split
   across scalar and vector engines with the 1/den fold.

Shapes (hardcoded): B=4, S=4096, D=1024, K=64.  L = S/2 = 2048 tokens
per core, processed in 16 chunks of C=128.
"""

import numpy as np

B, S, D, K = 4, 4096, 1024, 64
L = S // 2          # tokens per core (main), also prefix length
C = 128             # chunk (tokens)
G = 512             # token group for K-major projection matmuls
NCHUNK = L // C     # 16
NGRP = L // G       # 4
NDC = D // 128      # 8 contraction chunks
VS = 1.0 / 16.0     # V scale folded into num and den

_cache = {}


def _build_nc():
    import concourse.bacc as bacc
    import concourse.tile as tile
    from concourse import mybir

    f32 = mybir.dt.float32
    bf16 = mybir.dt.bfloat16
    AF = mybir.ActivationFunctionType
    Alu = mybir.AluOpType

    nc = bacc.Bacc()

    xtm = nc.declare_dram_parameter("xtm", [D, L], bf16, isOutput=False)
    xtp = nc.declare_dram_parameter("xtp", [D, L], bf16, isOutput=False)
    # wcat columns: [wqk 8*128 | wkv 8*128 | wv 8*64 | mask 128 | ident 64]
    WQK0, WKV0, WV0 = 0, NDC * 128, 2 * NDC * 128
    MSK0 = WV0 + NDC * 64
    ID0 = MSK0 + C
    WCOLS = ID0 + 64
    wcat = nc.declare_dram_parameter("wcat", [128, WCOLS], bf16, isOutput=False)
    wot = nc.declare_dram_parameter("wot", [K, D], bf16, isOutput=False)
    selc = nc.declare_dram_parameter("selc", [C, 1], f32, isOutput=False)
    out = nc.declare_dram_parameter("out", [L, D], bf16, isOutput=True)

    with tile.TileContext(nc) as tc:
        with (
            tc.tile_pool(name="consts", bufs=1) as consts,
            tc.tile_pool(name="xm", bufs=1) as xm_pool,
            tc.tile_pool(name="xp", bufs=1) as xp_pool,
            tc.tile_pool(name="small", bufs=4) as small,
            tc.tile_pool(name="vko", bufs=NCHUNK + 1) as vko_pool,
            tc.tile_pool(name="pvko", bufs=2) as pvko_pool,
            tc.tile_pool(name="qk", bufs=NCHUNK + 1) as qk_pool,
            tc.tile_pool(name="ks", bufs=NCHUNK + 2) as ks_pool,
            tc.tile_pool(name="tmp", bufs=4) as tmp_pool,
            tc.tile_pool(name="ptmp", bufs=4) as ptmp_pool,
            tc.tile_pool(name="ostage", bufs=3) as ostage_pool,
            tc.tile_pool(name="state_pool", bufs=1, space="PSUM") as state_pool,
        ):
            # ---- constants ----
            wcat_sb = consts.tile([128, WCOLS], bf16, tag="wcat")
            nc.sync.dma_start(out=wcat_sb, in_=wcat[:, :])
            wqk_sb = [wcat_sb[:, WQK0 + d * 128:WQK0 + (d + 1) * 128]
                      for d in range(NDC)]
            wkv_sb = [wcat_sb[:, WKV0 + d * 128:WKV0 + (d + 1) * 128]
                      for d in range(NDC)]
            wv_sb = [wcat_sb[:, WV0 + d * 64:WV0 + (d + 1) * 64]
                     for d in range(NDC)]
            mask_sb = wcat_sb[:, MSK0:MSK0 + C]
            ident_sb = wcat_sb[0:64, ID0:ID0 + 64]
            wot_sb = consts.tile([K, D], bf16, tag="wot")
            nc.sync.dma_start(out=wot_sb, in_=wot[:, :])
            selc_sb = consts.tile([C, 1], f32, tag="selc")
            nc.sync.dma_start(out=selc_sb, in_=selc[:, :])
            ones1_sb = consts.tile([1, 1], bf16, tag="ones1")
            nc.vector.memset(ones1_sb, 1.0)

            # ---- x tiles (resident); prefix sections first so the
            # prefix state is ready early, then main ----
            xp_all = [xp_pool.tile([128, L], bf16, name=f"xp{d}",
                                   tag=f"xp{d}") for d in range(NDC)]
            xm_all = [xm_pool.tile([128, L], bf16, name=f"xm{d}",
                                   tag=f"xm{d}") for d in range(NDC)]
            for g in range(NGRP):
                gs = slice(g * G, (g + 1) * G)
                for d in range(NDC):
                    nc.sync.dma_start(out=xp_all[d][:, gs],
                                      in_=xtp[d * 128:(d + 1) * 128, gs])
            for g in range(NGRP):
                gs = slice(g * G, (g + 1) * G)
                for d in range(NDC):
                    nc.sync.dma_start(out=xm_all[d][:, gs],
                                      in_=xtm[d * 128:(d + 1) * 128, gs])

            # running state [K, K+1]: cols 0:K = S'[k,m] (1/16-scaled),
            # col K = k_sum/16.
            state_ps = state_pool.tile([K, 1 + K], f32)

            # =============== PREFIX: token-major [K|V], state sum ======
            with tc.tile_pool(name="pp_ps", bufs=3, space="PSUM") as pp_pool:
                for ci in range(NCHUNK):
                    sl = slice(ci * C, (ci + 1) * C)
                    pp = pp_pool.tile([C, 2 * K], f32, tag="pp")
                    for d in range(NDC):
                        nc.tensor.matmul(pp, xp_all[d][:, sl], wkv_sb[d],
                                         start=(d == 0), stop=(d == NDC - 1))
                    eu = ptmp_pool.tile([C, K], f32, tag="eu")
                    nc.scalar.activation(eu, pp[:, 0:K], AF.Exp)
                    em = ptmp_pool.tile([C, K], f32, tag="em")
                    nc.vector.tensor_scalar_min(em, eu, 1.0)
                    pvko = pvko_pool.tile([C, 2 * K + 1], bf16, tag="pvko")
                    nc.vector.scalar_tensor_tensor(
                        pvko[:, 0:K], pp[:, 0:K], 1.0, em,
                        op0=Alu.add, op1=Alu.max)
                    # V and ones column scaled by sel/16 (zero on h=0)
                    nc.scalar.activation(pvko[:, K:2 * K], pp[:, K:2 * K],
                                         AF.Copy, scale=selc_sb)
                    nc.scalar.copy(pvko[:, 2 * K:2 * K + 1], selc_sb)
                    nc.tensor.matmul(state_ps, pvko[:, 0:K],
                                     pvko[:, K:2 * K + 1],
                                     start=(ci == 0), stop=False,
                                     skip_group_check=True)

            # prefix state snapshot (ks for chunk 0)
            ks_init = ks_pool.tile([K, 1 + K], bf16, tag="ks")
            nc.scalar.copy(ks_init, state_ps)

            # =============== MAIN projections =========================
            qTs, kTs, vkos = [], [], []
            with (
                tc.tile_pool(name="p1_ps", bufs=2, space="PSUM") as p1_pool,
                tc.tile_pool(name="pv_ps", bufs=2, space="PSUM") as pv_pool,
                tc.tile_pool(name="tk_ps", bufs=2, space="PSUM") as tk_pool,
            ):
                for g in range(NGRP):
                    gs = slice(g * G, (g + 1) * G)
                    xg = [t[:, gs] for t in xm_all]
                    p1g = p1_pool.tile([2 * K, G], f32, tag="p1")
                    for d in range(NDC):
                        nc.tensor.matmul(p1g, wqk_sb[d], xg[d],
                                         start=(d == 0), stop=(d == NDC - 1))
                    eg = tmp_pool.tile([2 * K, G], f32, tag="eg")
                    nc.scalar.activation(eg, p1g, AF.Exp)
                    eg2 = tmp_pool.tile([2 * K, G], f32, tag="eg2")
                    nc.vector.tensor_scalar_min(eg2, eg, 1.0)
                    for c4 in range(G // C):
                        ci = g * (G // C) + c4
                        sl = slice(c4 * C, (c4 + 1) * C)
                        qT = qk_pool.tile([K, C], bf16, tag="qT")
                        kT = qk_pool.tile([K, C], bf16, tag="kT")
                        nc.vector.scalar_tensor_tensor(
                            qT, p1g[0:K, sl], 1.0, eg2[0:K, sl],
                            op0=Alu.add, op1=Alu.max)
                        nc.vector.scalar_tensor_tensor(
                            kT, p1g[K:2 * K, sl], 1.0, eg2[K:2 * K, sl],
                            op0=Alu.add, op1=Alu.max)
                        qTs.append(qT)
                        kTs.append(kT)
                        # V token-major (free dim 64)
                        msl = slice(ci * C, (ci + 1) * C)
                        ppv = pv_pool.tile([C, K], f32, tag="ppv")
                        for d in range(NDC):
                            nc.tensor.matmul(ppv, xm_all[d][:, msl], wv_sb[d],
                                             start=(d == 0),
                                             stop=(d == NDC - 1))
                        # k token-major via PE transpose of kT
                        tkp = tk_pool.tile([C, K], bf16, tag="tkp")
                        nc.tensor.transpose(tkp, kT, ident_sb)
                        vko = vko_pool.tile([C, 2 * K + 1], bf16, tag="vko")
                        nc.scalar.copy(vko[:, 0:K], tkp)
                        nc.scalar.activation(vko[:, K:2 * K], ppv,
                                             AF.Copy, scale=VS)
                        nc.vector.memset(vko[:, 2 * K:2 * K + 1], VS)
                        vkos.append(vko)

            # =============== PHASE B: attention =======================
            with (
                tc.tile_pool(name="atnd_ps", bufs=5, space="PSUM") as atnd_pool,
                tc.tile_pool(name="op_ps", bufs=2, space="PSUM") as op_pool,
            ):
                ks_prev = ks_init
                for ci in range(NCHUNK):
                    qT, kT, vko = qTs[ci], kTs[ci], vkos[ci]
                    # intra-chunk scores A^T[t, s], causal mask
                    at = atnd_pool.tile([C, C], f32, tag="atnd")
                    nc.tensor.matmul(at, kT, qT, start=True, stop=True)
                    atm = tmp_pool.tile([C, C], bf16, tag="atm")
                    nc.vector.tensor_tensor(atm, at, mask_sb, Alu.mult)
                    # [num'^T ; den'] = [v'|c]^T A + S'^T q
                    nd = atnd_pool.tile([1 + K, C], f32, tag="atnd")
                    nc.tensor.matmul(nd, vko[:, K:2 * K + 1], atm,
                                     start=True, stop=False)
                    nc.tensor.matmul(nd, ks_prev, qT, start=False, stop=True)
                    # state update + snapshot
                    nc.tensor.matmul(state_ps, vko[:, 0:K],
                                     vko[:, K:2 * K + 1],
                                     start=False, stop=(ci == NCHUNK - 1),
                                     skip_group_check=True)
                    ks_i = ks_pool.tile([K, 1 + K], bf16, tag="ks")
                    nc.scalar.copy(ks_i, state_ps)
                    ks_prev = ks_i
                    # reciprocal of den' (transpose via 1-row matmul)
                    den_b = small.tile([1, C], bf16, tag="den")
                    nc.vector.tensor_scalar_add(den_b, nd[K:K + 1, :], 1e-6)
                    dtp = atnd_pool.tile([C, 1], f32, tag="atnd")
                    nc.tensor.matmul(dtp, den_b, ones1_sb, start=True,
                                     stop=True)
                    recip = small.tile([C, 1], f32, tag="recip")
                    nc.vector.reciprocal(recip, dtp)
                    # output projection; divide by den' on PSUM eviction
                    attn = qk_pool.tile([K, C], bf16, tag="attn")
                    nc.scalar.copy(attn, nd[0:K, :])
                    ost = ostage_pool.tile([C, D], bf16, tag="ost")
                    op0 = op_pool.tile([C, D // 2], f32, tag="op")
                    nc.tensor.matmul(op0, attn, wot_sb[:, 0:512],
                                     start=True, stop=True)
                    nc.scalar.activation(ost[:, 0:512], op0,
                                         AF.Copy, scale=recip)
                    op1 = op_pool.tile([C, D // 2], f32, tag="op")
                    nc.tensor.matmul(op1, attn, wot_sb[:, 512:1024],
                                     start=True, stop=True)
                    nc.vector.tensor_scalar_mul(ost[:, 512:1024], op1, recip)
                    nc.sync.dma_start(out=out[ci * C:(ci + 1) * C, :], in_=ost)

    nc.compile()
    worst = []
    for fn in nc.m.functions:
        for blk in fn.blocks:
            for inst in blk.instructions:
                n = len(inst.sync_info.on_wait) if inst.sync_info else 0
                if n > 1 and type(inst).__name__ == "InstMatmult":
                    worst.append((inst.name, n))
    if worst:
        print(f"WARNING: matmuls with >1 wait after lowering: {worst}")
    return nc


def _prep_inputs(x, Wq, Wk, Wv, Wo):
    import ml_dtypes

    bf16 = ml_dtypes.bfloat16
    wqk = np.concatenate([Wq.T, Wk.T], axis=1)                # [D, 2K]
    wkv = np.concatenate([Wk.T, Wv.T], axis=1)                # [D, 2K]
    wv = Wv.T                                                 # [D, K]
    mask = np.triu(np.ones((C, C), np.float32))               # keep t <= s
    ident = np.zeros((128, 64), np.float32)
    ident[0:64, 0:64] = np.eye(64)
    wcat = np.concatenate(
        [wqk[d * 128:(d + 1) * 128, :] for d in range(NDC)]
        + [wkv[d * 128:(d + 1) * 128, :] for d in range(NDC)]
        + [wv[d * 128:(d + 1) * 128, :] for d in range(NDC)]
        + [mask, ident],
        axis=1,
    ).astype(bf16)
    wot = np.ascontiguousarray(Wo.T).astype(bf16)             # [K, D]
    zeros_x = np.zeros((D, L), dtype=bf16)
    in_maps = []
    for core in range(8):
        b, h = core // 2, core % 2
        xb = np.ascontiguousarray(x[b].astype(bf16).T)        # [D, S]
        m = {
            "xtm": np.ascontiguousarray(xb[:, h * L:(h + 1) * L]),
            "xtp": np.ascontiguousarray(xb[:, 0:L]) if h else zeros_x,
            "wcat": wcat,
            "wot": wot,
            "selc": np.full((C, 1), float(h) / 16.0, np.float32),
        }
        in_maps.append(m)
    return in_maps


def _run(inputs, trace=False):
    from concourse.bass_utils import run_bass_kernel_spmd

    if "nc" not in _cache:
        _cache["nc"] = _build_nc()
    nc = _cache["nc"]
    in_maps = _prep_inputs(
        np.asarray(inputs["x"], np.float32),
        np.asarray(inputs["Wq"], np.float32),
        np.asarray(inputs["Wk"], np.float32),
        np.asarray(inputs["Wv"], np.float32),
        np.asarray(inputs["Wo"], np.float32),
    )
    res = run_bass_kernel_spmd(nc, in_maps, list(range(8)), trace=trace)
    out = np.empty((B, S, D), np.float32)
    for core in range(8):
        b, h = core // 2, core % 2
        out[b, h * L:(h + 1) * L, :] = res.results[core]["out"].astype(
            np.float32)
    return out, res


def kernel(**inputs) -> np.ndarray:
    out, _ = _run(inputs, trace=False)
    return out


# revision 10
# speedup vs baseline: 1.3476x; 1.1175x over previous
"""Low-rank linear attention (causal, elu+1 feature map) on 8 trn2 cores.

Sharding: core = 2*b + h  (batch b in 0..3, sequence half h in 0..1).
Each core computes out[b, h*2048:(h+1)*2048, :].  Second-half cores
recompute the running K^T.V state over their 2048-token prefix on device
(prefix V contributions are scaled by sel=h so one SPMD program serves
all 8 cores).

Structure (v3):
 - prefix: token-major [K|V] projection per 128-token chunk (PE stream,
   LDW hidden); all 16 rank-128 state updates emitted after the
   projections so the PE never stalls on eviction chains.
 - main: K-major [Q;K] in 512-token groups + token-major V-only
   (free dim 64) + per-chunk PE transpose of kT (emitted one group
   behind so kT is ready).
 - elu+1 = max(z+1, min(exp(z), 1)) on scalar+vector (no bias matmuls).
 - V carries 1/16 so num/den stay consistently scaled (ratio unchanged).
 - phase B software pipeline: scores two chunks ahead, output projection
   one chunk behind; den computed pre-transposed via two accumulating
   1-column matmuls; per-chunk state snapshots; PSUM evictions split
   scalar/vector/gpsimd; bf16 DRAM output.

Shapes (hardcoded): B=4, S=4096, D=1024, K=64.  L = S/2 = 2048 tokens
per core, processed in 16 chunks of C=128.
"""

import numpy as np

B, S, D, K = 4, 4096, 1024, 64
L = S // 2          # tokens per core (main), also prefix length
C = 128             # chunk (tokens)
G = 512             # token group for K-major projection matmuls
NCHUNK = L // C     # 16
NGRP = L // G       # 4
NDC = D // 128      # 8 contraction chunks
VS = 1.0 / 16.0     # V scale folded into num and den

_cache = {}


def _build_nc():
    import concourse.bacc as bacc
    import concourse.tile as tile
    from concourse import mybir

    f32 = mybir.dt.float32
    bf16 = mybir.dt.bfloat16
    AF = mybir.ActivationFunctionType
    Alu = mybir.AluOpType

    nc = bacc.Bacc()

    xtm = nc.declare_dram_parameter("xtm", [D, L], bf16, isOutput=False)
    xtp = nc.declare_dram_parameter("xtp", [D, L], bf16, isOutput=False)
    # wcat columns: [wqk 8*128 | wkv 8*128 | wv 8*64 | mask 128 | ident 64]
    WQK0, WKV0, WV0 = 0, NDC * 128, 2 * NDC * 128
    MSK0 = WV0 + NDC * 64
    ID0 = MSK0 + C
    WCOLS = ID0 + 64
    wcat = nc.declare_dram_parameter("wcat", [128, WCOLS], bf16, isOutput=False)
    wot = nc.declare_dram_parameter("wot", [K, D], bf16, isOutput=False)
    selc = nc.declare_dram_parameter("selc", [C, 1], f32, isOutput=False)
    out = nc.declare_dram_parameter("out", [L, D], bf16, isOutput=True)

    with tile.TileContext(nc) as tc:
        with (
            tc.tile_pool(name="consts", bufs=1) as consts,
            tc.tile_pool(name="xm", bufs=1) as xm_pool,
            tc.tile_pool(name="xp", bufs=1) as xp_pool,
            tc.tile_pool(name="small", bufs=4) as small,
            tc.tile_pool(name="vko", bufs=NCHUNK + 1) as vko_pool,
            tc.tile_pool(name="pvko", bufs=4) as pvko_pool,
            tc.tile_pool(name="qk", bufs=NCHUNK + 1) as qk_pool,
            tc.tile_pool(name="atn", bufs=4) as atn_pool,
            tc.tile_pool(name="ks", bufs=4) as ks_pool,
            tc.tile_pool(name="tmp", bufs=4) as tmp_pool,
            tc.tile_pool(name="ptmp", bufs=4) as ptmp_pool,
            tc.tile_pool(name="ostage", bufs=3) as ostage_pool,
            tc.tile_pool(name="state_pool", bufs=1, space="PSUM") as state_pool,
        ):
            # ---- constants ----
            wcat_sb = consts.tile([128, WCOLS], bf16, tag="wcat")
            nc.sync.dma_start(out=wcat_sb, in_=wcat[:, :])
            wqk_sb = [wcat_sb[:, WQK0 + d * 128:WQK0 + (d + 1) * 128]
                      for d in range(NDC)]
            wkv_sb = [wcat_sb[:, WKV0 + d * 128:WKV0 + (d + 1) * 128]
                      for d in range(NDC)]
            wv_sb = [wcat_sb[:, WV0 + d * 64:WV0 + (d + 1) * 64]
                     for d in range(NDC)]
            mask_sb = wcat_sb[:, MSK0:MSK0 + C]
            ident_sb = wcat_sb[0:64, ID0:ID0 + 64]
            wot_sb = consts.tile([K, D], bf16, tag="wot")
            nc.sync.dma_start(out=wot_sb, in_=wot[:, :])
            selc_sb = consts.tile([C, 1], f32, tag="selc")
            nc.sync.dma_start(out=selc_sb, in_=selc[:, :])
            # den-sum column: carries the same 1/16 scale as V's columns
            onec_sb = consts.tile([C, 1], bf16, tag="onec")
            nc.vector.memset(onec_sb, VS)

            # ---- x tiles (resident); prefix sections first ----
            xp_all = [xp_pool.tile([128, L], bf16, name=f"xp{d}",
                                   tag=f"xp{d}") for d in range(NDC)]
            xm_all = [xm_pool.tile([128, L], bf16, name=f"xm{d}",
                                   tag=f"xm{d}") for d in range(NDC)]
            for g in range(NGRP):
                gs = slice(g * G, (g + 1) * G)
                for d in range(NDC):
                    nc.sync.dma_start(out=xp_all[d][:, gs],
                                      in_=xtp[d * 128:(d + 1) * 128, gs])
            for g in range(NGRP):
                gs = slice(g * G, (g + 1) * G)
                for d in range(NDC):
                    nc.sync.dma_start(out=xm_all[d][:, gs],
                                      in_=xtm[d * 128:(d + 1) * 128, gs])

            # running state [K, K+1]: cols 0:K = S'[k,m] (1/16-scaled),
            # col K = k_sum/16.
            state_ps = state_pool.tile([K, 1 + K], f32)

            # =============== PREFIX: token-major [K|V], state sum ======
            pvkos = []
            with tc.tile_pool(name="pp_ps", bufs=4, space="PSUM") as pp_pool:
                for ci in range(NCHUNK):
                    sl = slice(ci * C, (ci + 1) * C)
                    pp = pp_pool.tile([C, 2 * K], f32, tag="pp")
                    for d in range(NDC):
                        nc.tensor.matmul(pp, xp_all[d][:, sl], wkv_sb[d],
                                         start=(d == 0), stop=(d == NDC - 1))
                    eu = ptmp_pool.tile([C, K], f32, tag="eu")
                    nc.scalar.activation(eu, pp[:, 0:K], AF.Exp)
                    em = ptmp_pool.tile([C, K], f32, tag="em")
                    nc.vector.tensor_scalar_min(em, eu, 1.0)
                    pvko = pvko_pool.tile([C, 2 * K + 1], bf16, tag="pvko",
                                          bufs=NCHUNK)
                    nc.vector.scalar_tensor_tensor(
                        pvko[:, 0:K], pp[:, 0:K], 1.0, em,
                        op0=Alu.add, op1=Alu.max)
                    # V and ones column scaled by sel/16 (zero on h=0)
                    nc.scalar.activation(pvko[:, K:2 * K], pp[:, K:2 * K],
                                         AF.Copy, scale=selc_sb)
                    nc.gpsimd.tensor_copy(pvko[:, 2 * K:2 * K + 1], selc_sb)
                    pvkos.append(pvko)
                # state updates after all projections: PE never stalls
                for ci in range(NCHUNK):
                    pvko = pvkos[ci]
                    nc.tensor.matmul(state_ps, pvko[:, 0:K],
                                     pvko[:, K:2 * K + 1],
                                     start=(ci == 0), stop=False,
                                     skip_group_check=True)

            # prefix state snapshot (ks for chunk 0)
            ks_init = ks_pool.tile([K, 1 + K], bf16, tag="ks")
            nc.scalar.copy(ks_init, state_ps)

            # =============== MAIN projections =========================
            qTs, kTs, vkos, ppvs = [], [], [], []
            with (
                tc.tile_pool(name="p1_ps", bufs=2, space="PSUM") as p1_pool,
                tc.tile_pool(name="pv_ps", bufs=2, space="PSUM") as pv_pool,
                tc.tile_pool(name="tk_ps", bufs=2, space="PSUM") as tk_pool,
            ):
                def transpose_k(ci):
                    """k token-major via PE transpose; finish vko[ci]."""
                    tkp = tk_pool.tile([C, K], bf16, tag="tkp")
                    nc.tensor.transpose(tkp, kTs[ci], ident_sb)
                    nc.scalar.copy(vkos[ci][:, 0:K], tkp)

                for g in range(NGRP):
                    gs = slice(g * G, (g + 1) * G)
                    p1g = p1_pool.tile([2 * K, G], f32, tag="p1")
                    for d in range(NDC):
                        nc.tensor.matmul(p1g, wqk_sb[d], xm_all[d][:, gs],
                                         start=(d == 0), stop=(d == NDC - 1))
                    eg = tmp_pool.tile([2 * K, G], f32, tag="eg", bufs=2)
                    nc.scalar.activation(eg, p1g, AF.Exp)
                    eg2 = tmp_pool.tile([2 * K, G], f32, tag="eg2", bufs=2)
                    nc.vector.tensor_scalar_min(eg2, eg, 1.0)
                    for c4 in range(G // C):
                        ci = g * (G // C) + c4
                        sl = slice(c4 * C, (c4 + 1) * C)
                        kT = qk_pool.tile([K, C], bf16, tag="kT")
                        nc.vector.scalar_tensor_tensor(
                            kT, p1g[K:2 * K, sl], 1.0, eg2[K:2 * K, sl],
                            op0=Alu.add, op1=Alu.max)
                        qT = qk_pool.tile([K, C], bf16, tag="qT")
                        nc.vector.scalar_tensor_tensor(
                            qT, p1g[0:K, sl], 1.0, eg2[0:K, sl],
                            op0=Alu.add, op1=Alu.max)
                        qTs.append(qT)
                        kTs.append(kT)
                        # V token-major (free dim 64)
                        msl = slice(ci * C, (ci + 1) * C)
                        ppv = pv_pool.tile([C, K], f32, tag="ppv")
                        for d in range(NDC):
                            nc.tensor.matmul(ppv, xm_all[d][:, msl], wv_sb[d],
                                             start=(d == 0),
                                             stop=(d == NDC - 1))
                        vko = vko_pool.tile([C, 2 * K + 1], bf16, tag="vko")
                        nc.scalar.activation(vko[:, K:2 * K], ppv,
                                             AF.Copy, scale=VS)
                        nc.gpsimd.memset(vko[:, 2 * K:2 * K + 1], VS)
                        vkos.append(vko)
                    # transposes one group behind (kT already evicted)
                    if g > 0:
                        for ci in range((g - 1) * 4, g * 4):
                            transpose_k(ci)
                for ci in range((NGRP - 1) * 4, NGRP * 4):
                    transpose_k(ci)

            # =============== PHASE B: attention =======================
            with (
                tc.tile_pool(name="sm_ps", bufs=5, space="PSUM") as sm_pool,
                tc.tile_pool(name="op_ps", bufs=2, space="PSUM") as op_pool,
            ):
                atms = {}

                def scores(ci):
                    at = sm_pool.tile([C, C], f32, tag="sm")
                    nc.tensor.matmul(at, kTs[ci], qTs[ci], start=True,
                                     stop=True)
                    atm = atn_pool.tile([C, C], bf16, tag="atm")
                    nc.vector.tensor_tensor(atm, at, mask_sb, Alu.mult)
                    atms[ci] = atm

                def outproj(ci, attn, recip):
                    ost = ostage_pool.tile([C, D], bf16, tag="ost")
                    op0 = op_pool.tile([C, D // 2], f32, tag="op")
                    nc.tensor.matmul(op0, attn, wot_sb[:, 0:512],
                                     start=True, stop=True)
                    op1 = op_pool.tile([C, D // 2], f32, tag="op")
                    nc.tensor.matmul(op1, attn, wot_sb[:, 512:1024],
                                     start=True, stop=True)
                    nc.scalar.activation(ost[:, 0:512], op0,
                                         AF.Copy, scale=recip)
                    nc.vector.tensor_scalar_mul(ost[:, 512:1024], op1, recip)
                    nc.sync.dma_start(out=out[ci * C:(ci + 1) * C, :],
                                      in_=ost)

                scores(0)
                scores(1)
                ks_prev = ks_init
                prev = None  # (ci, attn, recip) pending output projection
                for ci in range(NCHUNK):
                    qT, vko, atm = qTs[ci], vkos[ci], atms.pop(ci)
                    if prev is not None:
                        outproj(*prev)
                    if ci + 2 < NCHUNK:
                        scores(ci + 2)
                    # [num'^T] = v'^T A + S'^T q
                    nd = sm_pool.tile([K, C], f32, tag="sm")
                    nc.tensor.matmul(nd, vko[:, K:2 * K], atm,
                                     start=True, stop=False)
                    nc.tensor.matmul(nd, ks_prev[:, 0:K], qT,
                                     start=False, stop=True)
                    # den' transposed directly: [C,1]
                    den = sm_pool.tile([C, 1], f32, tag="sm")
                    nc.tensor.matmul(den, atm, onec_sb, start=True,
                                     stop=False)
                    nc.tensor.matmul(den, qT, ks_prev[:, K:K + 1],
                                     start=False, stop=True)
                    # state update + snapshot
                    nc.tensor.matmul(state_ps, vko[:, 0:K],
                                     vko[:, K:2 * K + 1],
                                     start=False, stop=(ci == NCHUNK - 1),
                                     skip_group_check=True)
                    ks_i = ks_pool.tile([K, 1 + K], bf16, tag="ks")
                    nc.scalar.copy(ks_i, state_ps)
                    ks_prev = ks_i
                    recip = small.tile([C, 1], f32, tag="recip")
                    nc.vector.reciprocal(recip, den)
                    attn = atn_pool.tile([K, C], bf16, tag="attn")
                    nc.scalar.copy(attn, nd)
                    prev = (ci, attn, recip)
                outproj(*prev)

    nc.compile()
    worst = []
    for fn in nc.m.functions:
        for blk in fn.blocks:
            for inst in blk.instructions:
                n = len(inst.sync_info.on_wait) if inst.sync_info else 0
                if n > 1 and type(inst).__name__ == "InstMatmult":
                    worst.append((inst.name, n))
    if worst:
        print(f"WARNING: matmuls with >1 wait after lowering: {worst}")
    return nc


def _prep_inputs(x, Wq, Wk, Wv, Wo):
    import ml_dtypes

    bf16 = ml_dtypes.bfloat16
    wqk = np.concatenate([Wq.T, Wk.T], axis=1)                # [D, 2K]
    wkv = np.concatenate([Wk.T, Wv.T], axis=1)                # [D, 2K]
    wv = Wv.T                                                 # [D, K]
    mask = np.triu(np.ones((C, C), np.float32))               # keep t <= s
    ident = np.zeros((128, 64), np.float32)
    ident[0:64, 0:64] = np.eye(64)
    wcat = np.concatenate(
        [wqk[d * 128:(d + 1) * 128, :] for d in range(NDC)]
        + [wkv[d * 128:(d + 1) * 128, :] for d in range(NDC)]
        + [wv[d * 128:(d + 1) * 128, :] for d in range(NDC)]
        + [mask, ident],
        axis=1,
    ).astype(bf16)
    wot = np.ascontiguousarray(Wo.T).astype(bf16)             # [K, D]
    zeros_x = np.zeros((D, L), dtype=bf16)
    in_maps = []
    for core in range(8):
        b, h = core // 2, core % 2
        xb = np.ascontiguousarray(x[b].astype(bf16).T)        # [D, S]
        m = {
            "xtm": np.ascontiguousarray(xb[:, h * L:(h + 1) * L]),
            "xtp": np.ascontiguousarray(xb[:, 0:L]) if h else zeros_x,
            "wcat": wcat,
            "wot": wot,
            "selc": np.full((C, 1), float(h) / 16.0, np.float32),
        }
        in_maps.append(m)
    return in_maps


def _run(inputs, trace=False):
    from concourse.bass_utils import run_bass_kernel_spmd

    if "nc" not in _cache:
        _cache["nc"] = _build_nc()
    nc = _cache["nc"]
    in_maps = _prep_inputs(
        np.asarray(inputs["x"], np.float32),
        np.asarray(inputs["Wq"], np.float32),
        np.asarray(inputs["Wk"], np.float32),
        np.asarray(inputs["Wv"], np.float32),
        np.asarray(inputs["Wo"], np.float32),
    )
    res = run_bass_kernel_spmd(nc, in_maps, list(range(8)), trace=trace)
    out = np.empty((B, S, D), np.float32)
    for core in range(8):
        b, h = core // 2, core % 2
        out[b, h * L:(h + 1) * L, :] = res.results[core]["out"].astype(
            np.float32)
    return out, res


def kernel(**inputs) -> np.ndarray:
    out, _ = _run(inputs, trace=False)
    return out
